# revision 8
# baseline (speedup 1.0000x reference)
"""DGN (graph attention network) forward pass on 8 Trainium2 NeuronCores.

Strategy: pure data parallelism over 128 independent graphs (16/core,
weights replicated). Activations are feature-major ([feature -> SBUF
partitions, node -> free dim]); weight-stationary matmuls span graph
PAIRS (moving width 512) to amortize LDWEIGHTS.

Attention redesign vs v1 (449us):
- Mask folded into exp on the Scalar engine: scores PSUM banks are
  seeded with 16*maskT via 4 concurrent diagonal-block matmuls
  (tile_position=(32b,32b)), the K=32 scores matmul accumulates on
  top, and exp(bias=-16) yields P = mask ? exp(s) : exp(s-16)~1e-7.
  This removes the per-head masked-exp multiply from the Vector
  engine entirely.
- q/k projections are natural-layout (head h at partitions 16h..16h+16);
  per-head score isolation comes from TWO zero-column-padded k weight
  copies (even/odd heads), so all partition bases stay 32-aligned.
- Scores run 4 heads at a time into 4 separate PSUM banks with 4-way
  row-group concurrency; exp processes all 4 banks in one ACTIVATE.
- AV is flipped: stationary = per-head [128,32] v-slices (16 v dims +
  16 ones columns for the denominator), moving = P, outputs col-tiled
  4-way into one PSUM bank. Kills the v1 LDWEIGHTS-bound AV deltas.
- Softmax denominators: full-tile reciprocal, then an SBUF->SBUF DMA
  broadcast of the den rows across each 32-band; attention rows are
  normalized with one tensor_tensor; the +v residual and the sparse
  head layout are absorbed into three Wo matmul terms (wo_even,
  wo_odd_shifted, wo_dense@v).
- Q head is flipped to qw-stationary with 4-way col-tiled partial
  sums combined by one [128,32] selector matmul.
"""

import os
import sys

for _p in ("/opt/trn_rl_repo",):
    if _p not in sys.path and os.path.isdir(_p):
        sys.path.append(_p)

import numpy as np

import concourse.bass as bass
import concourse.bacc as bacc
import concourse.tile as tile
from concourse import mybir
from concourse.masks import make_identity

F32 = mybir.dt.float32
BF16 = mybir.dt.bfloat16
I32 = mybir.dt.int32

B = 128          # total graphs
NCORES = 8
G = B // NCORES  # graphs per core
N = 256          # nodes per graph
NT = N // 128    # node tiles
F_IN = 128
HID = 512
KT = HID // 128  # K tiles over hidden dim
H = 8            # heads
D = 16           # head dim
HD = H * D       # 128
A = 32           # num actions
SCALE = 1.0 / (D ** 0.5)
MB = 16.0        # mask bias magnitude (exp(-16) ~ 1.1e-7)

WEIGHT_NAMES = [
    "enc_W1", "enc_b1", "enc_W2", "enc_b2",
    "Wv1", "bv1", "Wk1", "bk1", "Wq1", "bq1", "Wo1", "bo1",
    "Wv2", "bv2", "Wk2", "bk2", "Wq2", "bq2", "Wo2", "bo2",
    "q_W", "q_b",
]

Relu = mybir.ActivationFunctionType.Relu
Exp = mybir.ActivationFunctionType.Exp
AluOp = mybir.AluOpType


def _emit(nc, tc, ap, g_count):
    import contextlib
    ctx = contextlib.ExitStack()
    with ctx:
        # ---------------- pools (PSUM order fixes bank alignment) ----
        psc = ctx.enter_context(tc.tile_pool(name="psc", bufs=1, space="PSUM"))  # 4 banks
        pav = ctx.enter_context(tc.tile_pool(name="pav", bufs=2, space="PSUM"))  # 2 banks
        pmm = ctx.enter_context(tc.tile_pool(name="pmm", bufs=2, space="PSUM"))  # 2 banks

        wp = ctx.enter_context(tc.tile_pool(name="wp", bufs=1))       # persistent
        stg = ctx.enter_context(tc.tile_pool(name="stg", bufs=2))     # f32 staging
        gio = ctx.enter_context(tc.tile_pool(name="gio", bufs=6))     # per-graph dma-in
        act = ctx.enter_context(tc.tile_pool(name="act", bufs=3))     # h tensors
        sml = ctx.enter_context(tc.tile_pool(name="sml", bufs=4))     # per-use tiles
        esp = ctx.enter_context(tc.tile_pool(name="esp", bufs=3))     # exp tiles

        # ---------------- constants ----------------
        eye = wp.tile([128, 128], BF16, tag="eye")
        make_identity(nc, eye)
        eye16 = wp.tile([128, 128], BF16, tag="eye16")
        nc.vector.tensor_scalar(out=eye16, in0=eye, scalar1=MB, scalar2=0.0,
                                op0=AluOp.mult, op1=AluOp.add)
        eyef = wp.tile([128, 128], F32, tag="eyef")
        make_identity(nc, eyef)
        nmb = wp.tile([128, 1], F32, tag="nmb")
        nc.vector.memset(nmb, -MB)
        # sel4[32j+a, a] = 1  (Q-head partial-sum combiner)
        sel4 = wp.tile([128, A], BF16, tag="sel4")
        for j in range(4):
            nc.vector.tensor_copy(out=sel4[32 * j: 32 * j + 32, :],
                                  in_=eye[32 * j: 32 * j + 32, 32 * j: 32 * j + 32])

        # selT[32j+16, 32j+c]=1 for c in 0..17 (den broadcast selector)
        selA = stg.tile([128, 128], BF16, tag="selA")
        nc.gpsimd.memset(selA, 1.0)
        nc.gpsimd.affine_select(out=selA, in_=selA, compare_op=AluOp.is_equal,
                                fill=0.0, base=-16, pattern=[[1, 128]],
                                channel_multiplier=-32)
        selB = stg.tile([128, 128], BF16, tag="selB")
        nc.gpsimd.memset(selB, 1.0)
        nc.gpsimd.affine_select(out=selB, in_=selB, compare_op=AluOp.is_ge,
                                fill=0.0, base=0, pattern=[[1, 128]],
                                channel_multiplier=-32)
        nc.gpsimd.affine_select(out=selB, in_=selB, compare_op=AluOp.is_ge,
                                fill=0.0, base=16, pattern=[[-1, 128]],
                                channel_multiplier=32)
        sel_ps = pmm.tile([128, 128], F32, tag="mm", padded_shape=[128, 512])
        nc.tensor.matmul(sel_ps, selA[0:4, :], selB[0:4, :], start=True, stop=True)
        selT = wp.tile([128, 128], BF16, tag="selT")
        nc.vector.tensor_copy(out=selT, in_=sel_ps)

        _cast_engs = [nc.vector, nc.gpsimd, nc.scalar]
        _cast_i = [0]

        def cast_to(dst, src):
            eng = _cast_engs[_cast_i[0] % 3]
            _cast_i[0] += 1
            if eng is nc.scalar:
                eng.copy(out=dst, in_=src)
            else:
                eng.tensor_copy(out=dst, in_=src)

        def load_cast(name, src_ap, shape):
            st = stg.tile(shape, F32, tag="stage")
            nc.sync.dma_start(out=st, in_=src_ap)
            wt = wp.tile(shape, BF16, tag=name)
            cast_to(wt, st)
            return wt

        # encoder + q head weights (lhsT layouts)
        w1 = load_cast("w1", ap["enc_W1"], [128, HID])
        w2 = load_cast("w2", ap["enc_W2"].rearrange("(k p) m -> p k m", p=128), [128, KT, HID])
        qw = load_cast("qw", ap["q_W"].rearrange("(k p) m -> p k m", p=128), [128, 3 * KT, A])

        def load_bias_fm(name, n_mt):
            bt = wp.tile([128, n_mt], F32, tag="b_" + name)
            nc.sync.dma_start(out=bt, in_=ap[name].rearrange("(m p) -> p m", p=128))
            return bt

        b1 = load_bias_fm("enc_b1", KT)
        b2 = load_bias_fm("enc_b2", KT)

        # q_b / 4 replicated along partition bands
        qb4 = wp.tile([128, 1], F32, tag="qb4")
        for j in range(4):
            nc.sync.dma_start(out=qb4[32 * j: 32 * j + 32, :],
                              in_=ap["q_b"].rearrange("(p o) -> p o", o=1))
        nc.scalar.mul(out=qb4, in_=qb4, mul=0.25)

        layers = []
        for li in (1, 2):
            wv = load_cast(f"wv{li}", ap[f"Wv{li}"].rearrange("(k p) m -> p k m", p=128), [128, KT, HD])
            bv = wp.tile([128, 1], F32, tag=f"bv{li}")
            nc.sync.dma_start(out=bv, in_=ap[f"bv{li}"].rearrange("(p o) -> p o", o=1))

            wq = load_cast(f"wq{li}", ap[f"Wq{li}"].rearrange("(k p) m -> p k m", p=128), [128, KT, HD])
            bqs = wp.tile([128, 1], F32, tag=f"bq{li}")
            nc.sync.dma_start(out=bqs, in_=ap[f"bq{li}"].rearrange("(p o) -> p o", o=1))
            nc.scalar.mul(out=bqs, in_=bqs, mul=SCALE)

            # k: two zero-column-padded copies (even / odd heads)
            wk_st = stg.tile([128, KT, HD], F32, tag="stage")
            nc.sync.dma_start(out=wk_st, in_=ap[f"Wk{li}"].rearrange("(k p) m -> p k m", p=128))
            wks, bks = [], []
            for par in range(2):
                wkp = wp.tile([128, KT, HD], BF16, tag=f"wk{li}{par}")
                cast_to(wkp, wk_st)
                nc.vector.memset(
                    wkp.rearrange("p k (j two d) -> p k j two d", two=2, d=D)[:, :, :, 1 - par, :], 0.0)
                wks.append(wkp)
                bkt = wp.tile([128, 1], F32, tag=f"bk{li}{par}")
                nc.vector.memset(bkt, 0.0)
                for j in range(4):
                    lo = 32 * j + 16 * par
                    nc.sync.dma_start(
                        out=bkt[lo: lo + D, :],
                        in_=ap[f"bk{li}"][lo: lo + D].rearrange("(d o) -> d o", o=1))
                bks.append(bkt)

            # wo variants: dense, even-rows (natural), odd-rows (shifted -16)
            wo_c = load_cast(f"woc{li}", ap[f"Wo{li}"], [128, HID])
            wo_sp = []
            for par in range(2):
                st = stg.tile([128, HID], F32, tag="stage")
                nc.vector.memset(st, 0.0)
                for j in range(4):
                    nc.sync.dma_start(
                        out=st[32 * j: 32 * j + D, :],
                        in_=ap[f"Wo{li}"][32 * j + 16 * par: 32 * j + 16 * par + D, :])
                wt = wp.tile([128, HID], BF16, tag=f"wo{li}{par}")
                cast_to(wt, st)
                wo_sp.append(wt)
            bo = load_bias_fm(f"bo{li}", KT)
            layers.append(dict(wv=wv, bv=bv, wq=wq, bqs=bqs, wks=wks, bks=bks,
                               wo_c=wo_c, wo_a=wo_sp[0], wo_b=wo_sp[1], bo=bo))

        # ---------------- per-pair program ----------------
        def pair_prog(gs):
            ng = len(gs)
            W = N * ng

            # ---- loads + mask/x prep ----
            mT_l = []
            xq = []
            for g in gs:
                x_st = gio.tile([128, NT, F_IN], F32, tag="x")
                nc.sync.dma_start(out=x_st, in_=ap["x"][g].rearrange("(t p) f -> p t f", p=128))
                m_i = gio.tile([128, NT, N], I32, tag="mi")
                nc.sync.dma_start(out=m_i, in_=ap["mask"][g].rearrange("(t p) k -> p t k", p=128))
                m_b = sml.tile([128, NT, N], BF16, tag="mb")
                nc.gpsimd.tensor_copy(out=m_b, in_=m_i)
                mT = sml.tile([128, NT, N], BF16, tag="mT", bufs=6)
                for kt in range(NT):
                    for qt in range(NT):
                        nc.sync.dma_start_transpose(
                            out=mT[:, kt, 128 * qt: 128 * (qt + 1)],
                            in_=m_b[:, qt, 128 * kt: 128 * (kt + 1)])
                mT_l.append(mT)
                xq.append(x_st)
            yield

            xT = sml.tile([128, ng, N], BF16, tag="xT")
            for gi, g in enumerate(gs):
                x_b = sml.tile([128, NT, F_IN], BF16, tag="xb")
                nc.gpsimd.tensor_copy(out=x_b, in_=xq[gi])
                for t in range(NT):
                    nc.sync.dma_start_transpose(
                        out=xT[:, gi, 128 * t: 128 * (t + 1)],
                        in_=x_b[:, t, :])
            yield

            # ---- encoder (pair-wide) ----
            h1 = act.tile([128, KT, ng, N], BF16, tag="h1", bufs=2)
            for mt in range(KT):
                ps = pmm.tile([128, ng, N], F32, tag="mm")
                nc.tensor.matmul(ps.rearrange("p g n -> p (g n)"),
                                 w1[:, 128 * mt: 128 * (mt + 1)],
                                 xT.rearrange("p g n -> p (g n)"),
                                 start=True, stop=True)
                nc.scalar.activation(out=h1[:, mt, :, :], in_=ps, func=Relu,
                                     bias=b1[:, mt: mt + 1], scale=1.0)
                if mt % 2 == 1:
                    yield
            h0 = act.tile([128, KT, ng, N], BF16, tag="h0")
            for mt in range(KT):
                ps = pmm.tile([128, ng, N], F32, tag="mm")
                for kt in range(KT):
                    nc.tensor.matmul(ps.rearrange("p g n -> p (g n)"),
                                     w2[:, kt, 128 * mt: 128 * (mt + 1)],
                                     h1[:, kt, :, :].rearrange("p g n -> p (g n)"),
                                     start=(kt == 0), stop=(kt == KT - 1))
                nc.scalar.activation(out=h0[:, mt, :, :], in_=ps, func=Relu,
                                     bias=b2[:, mt: mt + 1], scale=1.0)
                if mt % 2 == 1:
                    yield

            # ---- attention layers ----
            h_in = h0
            h_keep = [h0]
            for li in range(2):
                L = layers[li]

                # q projection (natural layout, bias*SCALE, scale=SCALE)
                ps_q = pmm.tile([128, ng, N], F32, tag="mm")
                for kt in range(KT):
                    nc.tensor.matmul(ps_q.rearrange("p g n -> p (g n)"),
                                     L["wq"][:, kt, :],
                                     h_in[:, kt, :, :].rearrange("p g n -> p (g n)"),
                                     start=(kt == 0), stop=(kt == KT - 1))
                qp = sml.tile([128, ng, N], BF16, tag="qp")
                nc.scalar.activation(out=qp, in_=ps_q, func=Relu,
                                     bias=L["bqs"][:, 0:1], scale=SCALE)

                # k projections (two zero-padded parities)
                kps = []
                for par in range(2):
                    ps_k = pmm.tile([128, ng, N], F32, tag="mm")
                    for kt in range(KT):
                        nc.tensor.matmul(ps_k.rearrange("p g n -> p (g n)"),
                                         L["wks"][par][:, kt, :],
                                         h_in[:, kt, :, :].rearrange("p g n -> p (g n)"),
                                         start=(kt == 0), stop=(kt == KT - 1))
                    kp = sml.tile([128, ng, N], BF16, tag=f"kp{par}")
                    nc.scalar.activation(out=kp, in_=ps_k, func=Relu,
                                         bias=L["bks"][par][:, 0:1], scale=1.0)
                    kps.append(kp)
                yield

                # v projection + per-graph v_ext (both parities)
                ps_v = pmm.tile([128, ng, N], F32, tag="mm")
                for kt in range(KT):
                    nc.tensor.matmul(ps_v.rearrange("p g n -> p (g n)"),
                                     L["wv"][:, kt, :],
                                     h_in[:, kt, :, :].rearrange("p g n -> p (g n)"),
                                     start=(kt == 0), stop=(kt == KT - 1))
                vfm = sml.tile([128, ng, N], BF16, tag="vfm")
                nc.vector.tensor_scalar(out=vfm, in0=ps_v,
                                        scalar1=L["bv"][:, 0:1], scalar2=1e-6,
                                        op0=AluOp.add, op1=AluOp.max)
                vx_l = []
                for gi in range(ng):
                    vT = sml.tile([128, NT, 128], BF16, tag="vT")
                    for t in range(NT):
                        nc.sync.dma_start_transpose(
                            out=vT[:, t, :], in_=vfm[:, gi, 128 * t: 128 * (t + 1)])
                    vx = sml.tile([128, 2, NT, 4, 2 * D], BF16, tag="vx", bufs=6)
                    for par in range(2):
                        nc.vector.tensor_copy(
                            out=vx.rearrange("p w t j (two d) -> p w t j two d", two=2)[:, par, :, :, 0, :],
                            in_=vT.rearrange("p t (j two d) -> p t j two d", two=2, d=D)[:, :, :, par, :])
                    nc.vector.memset(vx[:, :, :, :, D:2 * D], 1.0)
                    vx_l.append(vx)
                yield

                # scores waves + exp + AV + normalize, per graph
                attn = sml.tile([128, 2, ng, N], BF16, tag="attn")
                for gi in range(ng):
                    mT = mT_l[gi]
                    e_s = esp.tile([128, 2, 4, NT, N], BF16, tag="es")
                    av = pav.tile([128, 2, N], F32, tag="av")
                    for w in range(2):  # wave w: heads 2i+w
                        sc = psc.tile([128, 4, NT, N], F32, tag="sc")
                        # seed banks with MB*maskT (4-way diagonal blocks,
                        # both kt regions in one 512-col matmul)
                        for step in range(4):
                            for i in range(4):
                                b = (i + step) % 4
                                nc.tensor.matmul(
                                    sc[32 * b: 32 * b + 32, i, :, :].rearrange("p t q -> p (t q)"),
                                    eye16[32 * b: 32 * b + 32, 32 * b: 32 * b + 32],
                                    mT[32 * b: 32 * b + 32, :, :].rearrange("p t q -> p (t q)"),
                                    start=True, stop=False,
                                    tile_position=(32 * b, 32 * b))
                        # K=32 scores accumulate (zero-padded k isolates head)
                        for kt in range(NT):
                            for i in range(4):
                                band = 32 * i
                                nc.tensor.matmul(
                                    sc[:, i, kt, :],
                                    kps[w][band: band + 32, gi, 128 * kt: 128 * (kt + 1)],
                                    qp[band: band + 32, gi, :],
                                    start=False, stop=(kt == NT - 1),
                                    tile_position=(band, 0))
                        yield
                        nc.scalar.activation(out=e_s[:, w, :, :, :], in_=sc,
                                             func=Exp, bias=nmb[:, 0:1], scale=1.0)
                        # AV pass w (col-tiled 4-way into one bank)
                        for kt in range(NT):
                            for j in range(4):
                                nc.tensor.matmul(
                                    av[32 * j: 32 * j + 32, w, :],
                                    vx_l[gi][:, w, kt, j, :],
                                    e_s[:, w, j, kt, :],
                                    start=(kt == 0), stop=(kt == NT - 1),
                                    tile_position=(0, 32 * j))
                        yield
                    # normalize: approx recip, selector-matmul broadcast, multiply
                    rdf = sml.tile([128, 2, N], F32, tag="rdf")
                    nc.vector.reciprocal_approx_fast(out=rdf, in_=av)
                    rden = sml.tile([128, 2, N], BF16, tag="rden")
                    nc.vector.tensor_copy(out=rden, in_=rdf)
                    bcf = psc.tile([128, 4, NT, N], F32, tag="sc")
                    bc_ps = bcf[:, 0, :, :]
                    nc.tensor.matmul(bc_ps.rearrange("p w q -> p (w q)"), selT,
                                     rden.rearrange("p w q -> p (w q)"),
                                     start=True, stop=True)
                    bc = sml.tile([128, 2, N], BF16, tag="bc")
                    nc.vector.tensor_copy(out=bc, in_=bc_ps)
                    nc.vector.tensor_mul(out=attn[:, :, gi, :], in0=av, in1=bc)
                    yield

                # output projection: wo_a@attnA + wo_b@attnB + wo_c@vfm
                h_out = act.tile([128, KT, ng, N], BF16, tag=f"hL{li}")
                for mt in range(KT):
                    ps2 = pmm.tile([128, ng, N], F32, tag="mm")
                    sl = slice(128 * mt, 128 * (mt + 1))
                    nc.tensor.matmul(ps2.rearrange("p g n -> p (g n)"),
                                     L["wo_a"][:, sl],
                                     attn[:, 0, :, :].rearrange("p g n -> p (g n)"),
                                     start=True, stop=False)
                    nc.tensor.matmul(ps2.rearrange("p g n -> p (g n)"),
                                     L["wo_b"][:, sl],
                                     attn[:, 1, :, :].rearrange("p g n -> p (g n)"),
                                     start=False, stop=False)
                    nc.tensor.matmul(ps2.rearrange("p g n -> p (g n)"),
                                     L["wo_c"][:, sl],
                                     vfm.rearrange("p g n -> p (g n)"),
                                     start=False, stop=True)
                    nc.scalar.activation(out=h_out[:, mt, :, :], in_=ps2, func=Relu,
                                         bias=L["bo"][:, mt: mt + 1], scale=1.0)
                    if mt % 2 == 1:
                        yield
                h_keep.append(h_out)
                h_in = h_out

            # ---- Q head: col-tiled partials + selector combine ----
            qh_ps = pmm.tile([128, ng, N], F32, tag="mm")
            for s in range(3):
                src = h_keep[s]
                for kt in range(KT):
                    nc.tensor.matmul(
                        qh_ps[32 * kt: 32 * kt + 32, :, :].rearrange("p g n -> p (g n)"),
                        qw[:, s * KT + kt, :],
                        src[:, kt, :, :].rearrange("p g n -> p (g n)"),
                        start=(s == 0), stop=(s == 2),
                        tile_position=(0, 32 * kt))
            qh_sb = sml.tile([128, ng, N], BF16, tag="qhsb")
            nc.vector.tensor_scalar_add(out=qh_sb, in0=qh_ps, scalar1=qb4[:, 0:1])
            qf_ps = pmm.tile([A, ng, N], F32, tag="mm")
            nc.tensor.matmul(qf_ps.rearrange("p g n -> p (g n)"), sel4,
                             qh_sb.rearrange("p g n -> p (g n)"),
                             start=True, stop=True)
            qf_sb = sml.tile([A, ng, N], F32, tag="qfsb")
            nc.vector.tensor_copy(out=qf_sb, in_=qf_ps)
            yield
            for gi, g in enumerate(gs):
                ps_f = pav.tile([128, NT, A], F32, tag="av",
                                padded_shape=[128, 2, N])
                for qt in range(NT):
                    nc.tensor.transpose(ps_f[:, qt, :],
                                        qf_sb[:, gi, 128 * qt: 128 * (qt + 1)],
                                        eyef[0:A, 0:A])
                o_sb = sml.tile([128, NT, A], F32, tag="osb")
                nc.vector.tensor_copy(out=o_sb, in_=ps_f)
                nc.sync.dma_start(out=ap["out"][g].rearrange("(t p) a -> p t a", p=128), in_=o_sb)
                yield

        # Drive pair generators with staggered starts (as v1).
        PIPE = 3
        STAGGER = 5
        pairs = [list(range(i, min(i + 2, g_count))) for i in range(0, g_count, 2)]
        active = [pair_prog(pairs.pop(0))]
        rounds = 0
        while pairs or active:
            rounds += 1
            if rounds % STAGGER == 0 and len(active) < PIPE and pairs:
                active.append(pair_prog(pairs.pop(0)))
            for gen in list(active):
                try:
                    next(gen)
                except StopIteration:
                    active.remove(gen)
                    if pairs:
                        active.append(pair_prog(pairs.pop(0)))


def build(g_count=G, num_devices=NCORES):
    nc = bacc.Bacc("TRN2", target_bir_lowering=False, debug=False,
                   num_devices=num_devices)
    ap = {}
    ap["x"] = nc.dram_tensor("x", [g_count, N, F_IN], F32, kind="ExternalInput").ap()
    ap["mask"] = nc.dram_tensor("mask", [g_count, N, N], I32, kind="ExternalInput").ap()
    shapes = {
        "enc_W1": [F_IN, HID], "enc_b1": [HID], "enc_W2": [HID, HID], "enc_b2": [HID],
        "q_W": [3 * HID, A], "q_b": [A],
    }
    for li in (1, 2):
        shapes[f"Wv{li}"] = [HID, HD]; shapes[f"bv{li}"] = [HD]
        shapes[f"Wk{li}"] = [HID, HD]; shapes[f"bk{li}"] = [HD]
        shapes[f"Wq{li}"] = [HID, HD]; shapes[f"bq{li}"] = [HD]
        shapes[f"Wo{li}"] = [HD, HID]; shapes[f"bo{li}"] = [HID]
    for nm in WEIGHT_NAMES:
        ap[nm] = nc.dram_tensor(nm, shapes[nm], F32, kind="ExternalInput").ap()
    ap["out"] = nc.dram_tensor("out", [g_count, N, A], F32, kind="ExternalOutput").ap()

    with tile.TileContext(nc) as tc:
        _emit(nc, tc, ap, g_count)
    nc.compile()
    return nc


_NC_CACHE = {}


def kernel(**inputs):
    key = "full"
    if key not in _NC_CACHE:
        _NC_CACHE[key] = build(G, NCORES)
    nc = _NC_CACHE[key]

    from concourse import bass_utils
    in_maps = []
    for c in range(NCORES):
        m = {
            "x": np.ascontiguousarray(inputs["x"][c * G:(c + 1) * G], dtype=np.float32),
            "mask": np.ascontiguousarray(inputs["mask"][c * G:(c + 1) * G], dtype=np.int32),
        }
        for nm in WEIGHT_NAMES:
            m[nm] = np.ascontiguousarray(inputs[nm], dtype=np.float32)
        in_maps.append(m)
    res = bass_utils.run_bass_kernel_spmd(nc, in_maps, core_ids=list(range(NCORES)))
    return np.concatenate([r["out"] for r in res.results], axis=0)
